# revision 15
# baseline (speedup 1.0000x reference)
"""Attention2d Trainium2 Bass kernel.

Reference computation (per batch element b of 8, one NeuronCore each):
    hn  = GroupNorm32(x) * gn1_scale + gn1_bias
    qkv = w_qkv @ hn + b_qkv          (1x1 conv == matmul over channels)
    per head h (8 heads, ch=64): q,k,v from qkv (torch reshape convention:
        head h uses rows h*192+{0..64,64..128,128..192})
    wgt = softmax((q*s)^T (k*s)), s = ch**-0.25
    a   = v @ wgt^T
    out = GroupNorm32(w_proj @ a + b_proj) ... * gn2_scale + gn2_bias
    y   = x + out

Device strategy (data-parallel over batch, 1 core per batch element):
  - channel-chunked layouts [128, 4, N] (partition = channel % 128-chunk)
  - S^T computed as k^T q in [s, t] layout so softmax sum folds into the
    second attention matmul via an appended ones-column on v^T
  - softmax without max-subtraction (logits are ~N(0, 0.2); exp is safe)
  - all big matmuls run as float32r (full PE rate)
  - GroupNorm group stats via tiny selection matmuls across partitions
"""

import numpy as np

NHEAD = 8
GROUPS = 32
EPS = 1e-5
B, C, H, W = 8, 512, 32, 32
N = H * W            # 1024 spatial positions
CH = C // NHEAD      # 64 channels per head
P = 128              # partitions
KC = C // P          # 4 channel chunks
NT = N // 512        # 2 column tiles of 512
SC = N // P          # 8 s-chunks

_CACHE = {}


def _build_nc(debug_taps=False):
    import concourse.bass as bass
    import concourse.tile as tile
    from concourse import mybir, bacc

    f32 = mybir.dt.float32
    f32r = mybir.dt.float32r
    AF = mybir.ActivationFunctionType
    OP = mybir.AluOpType

    nc = bacc.Bacc("TRN2", target_bir_lowering=False, num_devices=8)

    x_d = nc.dram_tensor("x", [P, KC, N], f32, kind="ExternalInput")
    wq_d = nc.dram_tensor("wq", [P, KC, C], f32r, kind="ExternalInput")
    wk_d = nc.dram_tensor("wk", [P, KC, C], f32r, kind="ExternalInput")
    wv_d = nc.dram_tensor("wv", [P, KC, C], f32r, kind="ExternalInput")
    wp_d = nc.dram_tensor("wp", [P, KC, C], f32r, kind="ExternalInput")
    bq_d = nc.dram_tensor("bq", [P, KC], f32, kind="ExternalInput")
    bk_d = nc.dram_tensor("bk", [P, KC], f32, kind="ExternalInput")
    bv_d = nc.dram_tensor("bv", [1, C], f32, kind="ExternalInput")
    bp_d = nc.dram_tensor("bp", [P, KC], f32, kind="ExternalInput")
    g1s_d = nc.dram_tensor("g1s", [P, KC], f32, kind="ExternalInput")
    g1b_d = nc.dram_tensor("g1b", [P, KC], f32, kind="ExternalInput")
    g2s_d = nc.dram_tensor("g2s", [P, KC], f32, kind="ExternalInput")
    g2b_d = nc.dram_tensor("g2b", [P, KC], f32, kind="ExternalInput")
    sel_d = nc.dram_tensor("sel", [P, KC, GROUPS], f32, kind="ExternalInput")
    selt_d = nc.dram_tensor("selt", [GROUPS, KC, P], f32, kind="ExternalInput")
    eh_d = nc.dram_tensor("eh", [NHEAD, KC, P], f32r, kind="ExternalInput")
    out_d = nc.dram_tensor("out", [P, KC, N], f32, kind="ExternalOutput")
    if debug_taps:
        dbg = {
            "d_hn": nc.dram_tensor("d_hn", [P, KC, N], f32, kind="ExternalOutput"),
            "d_q": nc.dram_tensor("d_q", [P, KC, N], f32, kind="ExternalOutput"),
            "d_k": nc.dram_tensor("d_k", [P, KC, N], f32, kind="ExternalOutput"),
            "d_vt": nc.dram_tensor("d_vt", [P, SC, NHEAD, CH + 1], f32, kind="ExternalOutput"),
            "d_exps0": nc.dram_tensor("d_exps0", [P, SC, N], f32, kind="ExternalOutput"),
            "d_sums0": nc.dram_tensor("d_sums0", [1, NHEAD, N], f32, kind="ExternalOutput"),
            "d_recip": nc.dram_tensor("d_recip", [NHEAD, N], f32, kind="ExternalOutput"),
            "d_au": nc.dram_tensor("d_au", [P, KC, N], f32, kind="ExternalOutput"),
            "d_proj": nc.dram_tensor("d_proj", [P, KC, N], f32, kind="ExternalOutput"),
        }

    with tile.TileContext(nc) as tc:
        with (
            tc.tile_pool(name="big", bufs=1) as big,
            tc.tile_pool(name="wpool", bufs=2) as wpool,
            tc.tile_pool(name="expp", bufs=2) as expp,
            tc.tile_pool(name="tmpp", bufs=2) as tmpp,
            tc.tile_pool(name="stp", bufs=2, space="PSUM") as stp,
            tc.tile_pool(name="qp", bufs=2, space="PSUM") as qp,
        ):
            # ---------- persistent SBUF tiles ----------
            x_sb = big.tile([P, KC, N], f32, tag="x_sb")
            hn = big.tile([P, KC, N], f32r, tag="hn")          # later reused as a_u
            q_sb = big.tile([P, KC, N], f32r, tag="q_sb")      # later reused as proj
            k_sb = big.tile([P, KC, N], f32r, tag="k_sb")
            vt = big.tile([P, SC, NHEAD, CH + 1], f32r, tag="vt")
            sums0 = big.tile([1, NHEAD, N], f32r, tag="sums0")
            sums = big.tile([NHEAD, N], f32r, tag="sums")
            bv_bc = big.tile([P, C], f32, tag="bv_bc")
            sel_sb = big.tile([P, KC, GROUPS], f32, tag="sel_sb")
            selt_sb = big.tile([GROUPS, KC, P], f32, tag="selt_sb")
            eh_sb = big.tile([NHEAD, KC, P], f32r, tag="eh_sb")
            bq_sb = big.tile([P, KC], f32, tag="bq_sb")
            bk_sb = big.tile([P, KC], f32, tag="bk_sb")
            bp_sb = big.tile([P, KC], f32, tag="bp_sb")
            bv_sb = big.tile([1, C], f32, tag="bv_sb")
            g1s = big.tile([P, KC], f32, tag="g1s")
            g1b = big.tile([P, KC], f32, tag="g1b")
            g2s = big.tile([P, KC], f32, tag="g2s")
            g2b = big.tile([P, KC], f32, tag="g2b")
            ones1 = big.tile([1, P], f32, tag="ones1")
            eps_sb = big.tile([GROUPS, 1], f32, tag="eps_sb")
            ab1 = big.tile([P, KC, 2], f32, tag="ab1")
            ab2 = big.tile([P, KC, 2], f32, tag="ab2")

            # ---------- input DMAs ----------
            for k in range(KC):
                nc.sync.dma_start(x_sb[:, k, :], x_d[:, k, :])
            wq_sb = wpool.tile([P, KC, C], f32r, tag="w")
            nc.sync.dma_start(wq_sb[:], wq_d[:])
            wk_sb = wpool.tile([P, KC, C], f32r, tag="w")
            nc.sync.dma_start(wk_sb[:], wk_d[:])
            for sb, d in (
                (bq_sb, bq_d), (bk_sb, bk_d), (bp_sb, bp_d), (bv_sb, bv_d),
                (g1s, g1s_d), (g1b, g1b_d), (g2s, g2s_d), (g2b, g2b_d),
                (sel_sb, sel_d), (selt_sb, selt_d), (eh_sb, eh_d),
            ):
                nc.sync.dma_start(sb[:], d[:])
            nc.vector.memset(ones1[:], 1.0)
            nc.vector.memset(eps_sb[:], EPS)
            onescol = big.tile([P, SC * NHEAD], f32, tag="onescol")
            nc.vector.memset(onescol[:], 1.0)
            nc.vector.tensor_copy(
                out=vt[:, :, :, CH : CH + 1],
                in_=onescol[:].rearrange("p (s h) -> p s h", h=NHEAD)[:, :, :, None],
            )

            # bv broadcast to all partitions via K=1 matmul
            ps_bv = qp.tile([P, N], f32, tag="qp")
            nc.tensor.matmul(ps_bv[:, 0:C], ones1[:], bv_sb[:], start=True, stop=True)
            nc.vector.tensor_copy(out=bv_bc[:], in_=ps_bv[:, 0:C])

            # ---------- GroupNorm helper ----------
            def group_norm_stats(src_tile, gs, gb, ab, uniq):
                """Compute per-channel (A, B) for y = src*A + B from group stats."""
                stat2 = big.tile([P, KC, 2], f32, tag=f"stat2{uniq}")
                for k in range(KC):
                    resh = src_tile[:, k, :].rearrange("p (s f) -> p s f", f=512)
                    stats = tmpp.tile([P, 2, 6], f32, tag="bnstats")
                    for s in range(2):
                        nc.vector.bn_stats(out=stats[:, s, :], in_=resh[:, s, :])
                    mv = tmpp.tile([P, 2], f32, tag="bnmv")
                    nc.vector.bn_aggr(out=mv[:], in_=stats[:])
                    # stat2 col0 = mean, col1 = E[x^2] = var + mean^2
                    nc.vector.tensor_copy(out=stat2[:, k, 0:1], in_=mv[:, 0:1])
                    musq = tmpp.tile([P, 1], f32, tag="musq")
                    nc.vector.tensor_tensor(musq[:], mv[:, 0:1], mv[:, 0:1], OP.mult)
                    nc.vector.tensor_tensor(stat2[:, k, 1:2], mv[:, 1:2], musq[:], OP.add)
                ps_g = qp.tile([P, N], f32, tag="qp")
                for k in range(KC):
                    nc.tensor.matmul(
                        ps_g[0:GROUPS, 0:2], sel_sb[:, k, :], stat2[:, k, :],
                        start=(k == 0), stop=(k == KC - 1),
                    )
                # group mean / E2 -> rstd = exp(-0.5*ln(var+eps))
                gstat = big.tile([GROUPS, 2], f32, tag=f"gstat{uniq}")
                gms = tmpp.tile([GROUPS, 2], f32, tag="gms")
                nc.vector.tensor_copy(out=gms[:], in_=ps_g[0:GROUPS, 0:2])
                nc.vector.tensor_copy(out=gstat[:, 0:1], in_=gms[:, 0:1])
                gvar = tmpp.tile([GROUPS, 1], f32, tag="gvar")
                gmusq = tmpp.tile([GROUPS, 1], f32, tag="gmusq")
                nc.vector.tensor_tensor(gmusq[:], gms[:, 0:1], gms[:, 0:1], OP.mult)
                nc.vector.tensor_tensor(gvar[:], gms[:, 1:2], gmusq[:], OP.subtract)
                nc.scalar.activation(out=gvar[:], in_=gvar[:], func=AF.Ln, bias=eps_sb[:])
                nc.scalar.activation(out=gstat[:, 1:2], in_=gvar[:], func=AF.Exp, scale=-0.5)
                # broadcast (mean, rstd) back to channels; A = scale*rstd, B = bias - mean*A
                for k in range(KC):
                    ps_c = qp.tile([P, N], f32, tag="qp")
                    nc.tensor.matmul(ps_c[:, 0:2], selt_sb[:, k, :], gstat[:], start=True, stop=True)
                    cst = tmpp.tile([P, 2], f32, tag="cst")
                    nc.vector.tensor_copy(out=cst[:], in_=ps_c[:, 0:2])
                    nc.vector.tensor_tensor(ab[:, k, 0:1], gs[:, k : k + 1], cst[:, 1:2], OP.mult)
                    ma = tmpp.tile([P, 1], f32, tag="ma")
                    nc.vector.tensor_tensor(ma[:], cst[:, 0:1], ab[:, k, 0:1], OP.mult)
                    nc.vector.tensor_tensor(ab[:, k, 1:2], gb[:, k : k + 1], ma[:], OP.subtract)

            # ---------- GN1 -> hn ----------
            group_norm_stats(x_sb, g1s, g1b, ab1, "1")
            for k in range(KC):
                nc.vector.tensor_scalar(
                    hn[:, k, :], x_sb[:, k, :],
                    ab1[:, k, 0:1], ab1[:, k, 1:2], OP.mult, OP.add,
                )

            # ---------- QKV ----------
            def conv_out(dst, w_sb, bias_sb, m):
                ps = qp.tile([P, N], f32, tag="qp")
                for t in range(NT):
                    for k in range(KC):
                        nc.tensor.matmul(
                            ps[:, t * 512 : (t + 1) * 512],
                            w_sb[:, k, m * P : (m + 1) * P],
                            hn[:, k, t * 512 : (t + 1) * 512],
                            start=(k == 0), stop=(k == KC - 1),
                        )
                nc.vector.tensor_scalar(
                    dst[:, m, :], ps[:], bias_sb[:, m : m + 1], None, OP.add,
                )

            for m in range(KC):
                conv_out(q_sb, wq_sb, bq_sb, m)
                conv_out(k_sb, wk_sb, bk_sb, m)

            wv_sb = wpool.tile([P, KC, C], f32r, tag="w")
            nc.sync.dma_start(wv_sb[:], wv_d[:])
            wp_sb = wpool.tile([P, KC, C], f32r, tag="w")
            nc.sync.dma_start(wp_sb[:], wp_d[:])

            # v^T tiles: [s-part, head-major channel], + bias broadcast
            for nt2 in range(SC // 2):
                ps = qp.tile([P, N], f32, tag="qp")
                for half in range(2):
                    nt = nt2 * 2 + half
                    for k in range(KC):
                        nc.tensor.matmul(
                            ps[:, half * C : half * C + C],
                            hn[:, k, nt * P : (nt + 1) * P],
                            wv_sb[:, k, :],
                            start=(k == 0), stop=(k == KC - 1),
                        )
                for half in range(2):
                    nt = nt2 * 2 + half
                    nc.vector.tensor_tensor(
                        vt[:, nt, :, 0:CH],
                        ps[:, half * C : half * C + C].rearrange("p (h c) -> p h c", h=NHEAD),
                        bv_bc[:].rearrange("p (h c) -> p h c", h=NHEAD),
                        OP.add,
                    )

            if debug_taps:
                nc.sync.dma_start(dbg["d_hn"][:], hn[:].bitcast(f32))
                nc.sync.dma_start(dbg["d_vt"][:], vt[:].bitcast(f32))

            # ---------- attention ----------
            a_u = hn  # hn is dead after the V matmuls; reuse as a_unnorm
            for h in range(NHEAD):
                p, e = h // 2, h % 2
                rows = slice(64 * e, 64 * e + 64)
                exps = expp.tile([P, SC, N], f32r, tag="exps")
                for sc in range(SC):
                    ps_st = stp.tile([P, N], f32, tag="stp")
                    for t in range(NT):
                        nc.tensor.matmul(
                            ps_st[:, t * 512 : (t + 1) * 512],
                            k_sb[rows, p, sc * P : (sc + 1) * P],
                            q_sb[rows, p, t * 512 : (t + 1) * 512],
                            start=True, stop=True,
                        )
                    nc.scalar.activation(out=exps[:, sc, :], in_=ps_st[:], func=AF.Exp)
                ps_a = qp.tile([P, N], f32, tag="qp")
                for t in range(NT):
                    for sc in range(SC):
                        nc.tensor.matmul(
                            ps_a[0 : CH + 1, t * 512 : (t + 1) * 512],
                            vt[:, sc, h, :],
                            exps[:, sc, t * 512 : (t + 1) * 512],
                            start=(sc == 0), stop=(sc == SC - 1),
                        )
                nc.vector.tensor_copy(out=a_u[rows, p, :], in_=ps_a[0:CH, :])
                nc.vector.tensor_copy(out=sums0[0:1, h, :], in_=ps_a[CH : CH + 1, :])
                if debug_taps and h == 0:
                    nc.sync.dma_start(dbg["d_q"][:], q_sb[:].bitcast(f32))
                    nc.sync.dma_start(dbg["d_k"][:], k_sb[:].bitcast(f32))
                    nc.sync.dma_start(dbg["d_exps0"][:], exps[:].bitcast(f32))

            # redistribute per-head sums rows onto 8 partitions, then
            # softmax denominators: recip = exp(-ln(sums))
            if debug_taps:
                nc.sync.dma_start(dbg["d_sums0"][:], sums0[:].bitcast(f32))
            nc.sync.dma_start(sums[:], sums0[0:1, :, :])
            nc.scalar.activation(out=sums[:], in_=sums[:], func=AF.Ln)
            recip = sums
            nc.scalar.activation(out=recip[:], in_=sums[:], func=AF.Exp, scale=-1.0)

            # normalize a: broadcast per-head recip rows to channel layout via matmul
            for p in range(KC):
                ps_rb = qp.tile([P, N], f32, tag="qp")
                for t in range(NT):
                    nc.tensor.matmul(
                        ps_rb[:, t * 512 : (t + 1) * 512],
                        eh_sb[:, p, :],
                        recip[:, t * 512 : (t + 1) * 512],
                        start=True, stop=True,
                    )
                nc.vector.tensor_tensor(a_u[:, p, :], a_u[:, p, :], ps_rb[:], OP.mult)

            if debug_taps:
                nc.sync.dma_start(dbg["d_recip"][:], recip[:].bitcast(f32))
                nc.sync.dma_start(dbg["d_au"][:], a_u[:].bitcast(f32))

            # ---------- proj ----------
            proj = q_sb  # q_sb dead after attention; reuse
            for m in range(KC):
                ps = qp.tile([P, N], f32, tag="qp")
                for t in range(NT):
                    for k in range(KC):
                        nc.tensor.matmul(
                            ps[:, t * 512 : (t + 1) * 512],
                            wp_sb[:, k, m * P : (m + 1) * P],
                            a_u[:, k, t * 512 : (t + 1) * 512],
                            start=(k == 0), stop=(k == KC - 1),
                        )
                nc.vector.tensor_scalar(
                    proj[:, m, :], ps[:], bp_sb[:, m : m + 1], None, OP.add,
                )

            if debug_taps:
                nc.sync.dma_start(dbg["d_proj"][:], proj[:].bitcast(f32))

            # ---------- GN2 + residual ----------
            group_norm_stats(proj, g2s, g2b, ab2, "2")
            for k in range(KC):
                nc.vector.tensor_scalar(
                    proj[:, k, :], proj[:, k, :],
                    ab2[:, k, 0:1], ab2[:, k, 1:2], OP.mult, OP.add,
                )
                nc.vector.tensor_tensor(x_sb[:, k, :], x_sb[:, k, :], proj[:, k, :], OP.add)
                nc.sync.dma_start(out_d[:, k, :], x_sb[:, k, :])

    nc.compile()
    return nc


def _host_prep(x, gn1_scale, gn1_bias, w_qkv, b_qkv, w_proj, b_proj, gn2_scale, gn2_bias):
    """Build per-core input maps (numpy only)."""
    f = np.float32
    x = np.asarray(x, f)
    w_qkv = np.asarray(w_qkv, f)
    b_qkv = np.asarray(b_qkv, f)
    w_proj = np.asarray(w_proj, f)
    b_proj = np.asarray(b_proj, f)
    gn1_scale = np.asarray(gn1_scale, f)
    gn1_bias = np.asarray(gn1_bias, f)
    gn2_scale = np.asarray(gn2_scale, f)
    gn2_bias = np.asarray(gn2_bias, f)

    def chunk_vec(v):  # [C] -> [P, KC]
        return np.ascontiguousarray(v.reshape(KC, P).T)

    def chunk_mat(wt):  # [C, M] -> [P, KC, M]
        return np.ascontiguousarray(wt.reshape(KC, P, -1).transpose(1, 0, 2))

    idx = np.arange(NHEAD)[:, None] * (3 * CH) + np.arange(CH)[None, :]
    q_idx, k_idx, v_idx = idx.ravel(), (idx + CH).ravel(), (idx + 2 * CH).ravel()

    s2 = float(CH) ** -0.5
    wq = chunk_mat(w_qkv[q_idx].T * s2)
    wk = chunk_mat(w_qkv[k_idx].T)
    wv = chunk_mat(w_qkv[v_idx].T)
    wp = chunk_mat(w_proj.T)
    bq = chunk_vec(b_qkv[q_idx] * s2)
    bk = chunk_vec(b_qkv[k_idx])
    bv = np.ascontiguousarray(b_qkv[v_idx].reshape(1, C))
    bp = chunk_vec(b_proj)

    cidx = np.arange(C)
    sel = np.zeros((P, KC, GROUPS), f)
    sel[cidx % P, cidx // P, cidx // 16] = 1.0 / 16.0
    selt = np.zeros((GROUPS, KC, P), f)
    selt[cidx // 16, cidx // P, cidx % P] = 1.0
    eh = np.zeros((NHEAD, KC, P), f)
    for k in range(KC):
        for c in range(P):
            eh[2 * k + c // CH, k, c] = 1.0

    shared = {
        "wq": wq, "wk": wk, "wv": wv, "wp": wp,
        "bq": bq, "bk": bk, "bv": bv, "bp": bp,
        "g1s": chunk_vec(gn1_scale), "g1b": chunk_vec(gn1_bias),
        "g2s": chunk_vec(gn2_scale), "g2b": chunk_vec(gn2_bias),
        "sel": sel, "selt": selt, "eh": eh,
    }
    in_maps = []
    for b in range(B):
        xb = np.ascontiguousarray(
            x[b].reshape(C, N).reshape(KC, P, N).transpose(1, 0, 2)
        )
        in_maps.append({"x": xb, **shared})
    return in_maps


def _assemble(results):
    out = np.empty((B, C, H, W), np.float32)
    for b in range(B):
        ob = np.asarray(results[b]["out"])  # [P, KC, N]
        out[b] = ob.transpose(1, 0, 2).reshape(C, N).reshape(C, H, W)
    return out


def get_nc():
    if "nc" not in _CACHE:
        _CACHE["nc"] = _build_nc()
    return _CACHE["nc"]


def kernel(x, gn1_scale, gn1_bias, w_qkv, b_qkv, w_proj, b_proj, gn2_scale, gn2_bias):
    from concourse.bass_utils import run_bass_kernel_spmd

    nc = get_nc()
    in_maps = _host_prep(
        x, gn1_scale, gn1_bias, w_qkv, b_qkv, w_proj, b_proj, gn2_scale, gn2_bias
    )
    res = run_bass_kernel_spmd(nc, in_maps, core_ids=list(range(B)))
    return _assemble(res.results)


# revision 50
# speedup vs baseline: 29.7804x; 29.7804x over previous
"""Attention2d Trainium2 Bass kernel.

Reference computation (per batch element b of 8, one NeuronCore each):
    hn  = GroupNorm32(x) * gn1_scale + gn1_bias
    qkv = w_qkv @ hn + b_qkv          (1x1 conv == matmul over channels)
    per head h (8 heads, ch=64): q,k,v from qkv (torch reshape convention:
        head h uses rows h*192+{0..64,64..128,128..192})
    wgt = softmax((q*s)^T (k*s)), s = ch**-0.25
    a   = v @ wgt^T
    out = GroupNorm32(w_proj @ a + b_proj) ... * gn2_scale + gn2_bias
    y   = x + out

Device strategy (data-parallel over batch, 1 core per batch element):
  - channel-chunked layouts [128, 4, N] (partition = channel % 128-chunk)
  - S^T computed as k^T q in [s, t] layout so the softmax denominator folds
    into the second attention matmul via an appended ones-column on v^T
  - softmax without max-subtraction (logits are ~N(0, 0.2); exp is safe)
  - all big matmuls run as float32r (full PE rate; fp32 is 4x slower)
  - GroupNorm group stats via tiny selection matmuls across partitions
  - emission is pair-interleaved (Q/K of a head pair, then its two heads'
    S^T/exp/A, then the pair's softmax denominators) so the ACT-bound
    attention pipeline starts as early as possible
"""

import numpy as np

NHEAD = 8
GROUPS = 32
EPS = 1e-5
B, C, H, W = 8, 512, 32, 32
N = H * W            # 1024 spatial positions
CH = C // NHEAD      # 64 channels per head
P = 128              # partitions
KC = C // P          # 4 channel chunks
NT = N // 512        # 2 column tiles of 512
SC = N // P          # 8 s-chunks

_CACHE = {}


def _build_nc(debug_taps=False):
    import concourse.tile as tile
    from concourse import mybir, bacc
    from concourse.hw_specs import get_activation_tables

    f32 = mybir.dt.float32
    f32r = mybir.dt.float32r
    AF = mybir.ActivationFunctionType
    OP = mybir.AluOpType

    nc = bacc.Bacc("TRN2", target_bir_lowering=False, num_devices=8)

    x_d = nc.dram_tensor("x", [P, KC, N], f32, kind="ExternalInput")
    wq_d = nc.dram_tensor("wq", [P, KC, KC, P], f32r, kind="ExternalInput")
    wk_d = nc.dram_tensor("wk", [P, KC, KC, P], f32r, kind="ExternalInput")
    wv_d = nc.dram_tensor("wv", [P, KC, C], f32r, kind="ExternalInput")
    wp_d = nc.dram_tensor("wp", [P, KC, C], f32r, kind="ExternalInput")
    bq_d = nc.dram_tensor("bq", [P, KC], f32, kind="ExternalInput")
    bk_d = nc.dram_tensor("bk", [P, KC], f32, kind="ExternalInput")
    bv_d = nc.dram_tensor("bv", [1, C], f32r, kind="ExternalInput")
    bp_d = nc.dram_tensor("bp", [P, KC], f32, kind="ExternalInput")
    g1s_d = nc.dram_tensor("g1s", [P, KC], f32, kind="ExternalInput")
    g1b_d = nc.dram_tensor("g1b", [P, KC], f32, kind="ExternalInput")
    g2s_d = nc.dram_tensor("g2s", [P, KC], f32, kind="ExternalInput")
    g2b_d = nc.dram_tensor("g2b", [P, KC], f32, kind="ExternalInput")
    sel_d = nc.dram_tensor("sel", [P, KC, GROUPS], f32, kind="ExternalInput")
    selt_d = nc.dram_tensor("selt", [GROUPS, KC, P], f32, kind="ExternalInput")
    eh_d = nc.dram_tensor("eh", [P, 2, P], f32r, kind="ExternalInput")
    out_d = nc.dram_tensor("out", [P, KC, N], f32, kind="ExternalOutput")
    if debug_taps:
        dbg = {
            "d_hn": nc.dram_tensor("d_hn", [P, KC, N], f32, kind="ExternalOutput"),
            "d_q": nc.dram_tensor("d_q", [P, KC, N], f32, kind="ExternalOutput"),
            "d_k": nc.dram_tensor("d_k", [P, KC, N], f32, kind="ExternalOutput"),
            "d_vt": nc.dram_tensor("d_vt", [P, SC, NHEAD, CH + 1], f32, kind="ExternalOutput"),
            "d_exps0": nc.dram_tensor("d_exps0", [P, SC, N], f32, kind="ExternalOutput"),
            "d_au": nc.dram_tensor("d_au", [P, KC, N], f32, kind="ExternalOutput"),
            "d_proj": nc.dram_tensor("d_proj", [P, KC, N], f32, kind="ExternalOutput"),
        }

    with tile.TileContext(nc) as tc:
        with (
            tc.tile_pool(name="big", bufs=1) as big,
            tc.tile_pool(name="wpool", bufs=3) as wpool,
            tc.tile_pool(name="qpool", bufs=2) as qpool,
            tc.tile_pool(name="kpool", bufs=4) as kpool,
            tc.tile_pool(name="vtp", bufs=1) as vtp,
            tc.tile_pool(name="expp", bufs=2) as expp,
            tc.tile_pool(name="tmpp", bufs=2) as tmpp,
            tc.tile_pool(name="stp", bufs=2, space="PSUM") as stp,
            tc.tile_pool(name="apool", bufs=2, space="PSUM") as apool,
            tc.tile_pool(name="qp", bufs=2, space="PSUM") as qp,
        ):
            # ---------- persistent SBUF tiles ----------
            x_sb = big.tile([P, KC, N], f32, tag="x_sb")
            hn = big.tile([P, KC, N], f32r, tag="hn")          # later reused as a_u
            vt = vtp.tile([P, SC, NHEAD, CH + 1], f32r, tag="vp")
            sums0 = big.tile([1, KC, N], f32r, tag="sums0")
            sums = big.tile([P, N], f32r, tag="sums")
            sel_sb = big.tile([P, KC, GROUPS], f32, tag="sel_sb")
            selt_sb = big.tile([GROUPS, KC, P], f32, tag="selt_sb")
            eh_sb = big.tile([P, 2, P], f32r, tag="eh_sb")
            bq_sb = big.tile([P, KC], f32, tag="bq_sb")
            bk_sb = big.tile([P, KC], f32, tag="bk_sb")
            bp_sb = big.tile([P, KC], f32, tag="bp_sb")
            bv_sb = big.tile([1, C], f32r, tag="bv_sb")
            g1s = big.tile([P, KC], f32, tag="g1s")
            g1b = big.tile([P, KC], f32, tag="g1b")
            g2s = big.tile([P, KC], f32, tag="g2s")
            g2b = big.tile([P, KC], f32, tag="g2b")
            ones1 = big.tile([1, P], f32r, tag="ones1")
            eps_sb = big.tile([GROUPS, 1], f32, tag="eps_sb")
            ab1 = big.tile([P, KC, 2], f32, tag="ab1")
            ab2 = big.tile([P, KC, 2], f32, tag="ab2")

            # ---------- input DMAs ----------
            for k in range(KC):
                for hlf in range(2):
                    nc.sync.dma_start(
                        x_sb[:, k, hlf * 512 : (hlf + 1) * 512],
                        x_d[:, k, hlf * 512 : (hlf + 1) * 512],
                    )
            for sb, d in (
                (sel_sb, sel_d), (selt_sb, selt_d), (g1s, g1s_d), (g1b, g1b_d),
                (bq_sb, bq_d), (bk_sb, bk_d), (bp_sb, bp_d), (bv_sb, bv_d),
                (g2s, g2s_d), (g2b, g2b_d), (eh_sb, eh_d),
            ):
                nc.sync.dma_start(sb[:], d[:])
            wq_sb = wpool.tile([P, KC, KC, P], f32r, tag="w")
            wk_sb = wpool.tile([P, KC, KC, P], f32r, tag="w")
            for m in range(KC):
                nc.sync.dma_start(wq_sb[:, m], wq_d[:, m])
                nc.sync.dma_start(wk_sb[:, m], wk_d[:, m])
            wv_sb = wpool.tile([P, KC, C], f32r, tag="w")
            nc.sync.dma_start(wv_sb[:], wv_d[:])

            # Preload the combined ln+exp ACT table set once, so the bacc
            # table-load pass doesn't thrash between natural_log and
            # exp_and_others at every Ln/Exp transition (~2.7us per switch).
            _set_names = list(get_activation_tables(nc.m.arch).keys())
            _tl = mybir.InstLoadActFuncSet(
                name=nc.get_next_instruction_name(),
                ins=[],
                outs=[],
                act_func_set_id=_set_names.index("natural_log_exp_and_others"),
            )
            _tl.engine = mybir.EngineType.Activation
            nc.scalar.add_instruction(_tl)

            nc.vector.memset(eps_sb[:], EPS)
            nc.vector.memset(sums[:].bitcast(f32), 1.0)
            onescol = big.tile([P, 1], f32, tag="onescol")
            nc.vector.memset(onescol[:], 1.0)
            nc.vector.tensor_copy(
                out=vt[:, :, :, CH : CH + 1],
                in_=onescol[:, :, None, None].to_broadcast((P, SC, NHEAD, 1)),
            )
            nc.vector.tensor_copy(
                out=ones1[:], in_=onescol[0:1, :].to_broadcast((1, P))
            )

            # ---------- GroupNorm helper ----------
            def group_norm_stats(src_tile, gs, gb, ab, uniq):
                """Per-channel (A, B) for y = src*A + B from 32-group stats."""
                stat2 = big.tile([P, KC, 2], f32, tag=f"stat2{uniq}")
                mvs = tmpp.tile([P, KC, 2], f32, tag="mvs")
                for k in range(KC):
                    resh = src_tile[:, k, :].rearrange("p (s f) -> p s f", f=512)
                    stats = tmpp.tile([P, 2, 6], f32, tag="bnstats")
                    for si in range(2):
                        nc.vector.bn_stats(out=stats[:, si, :], in_=resh[:, si, :])
                    nc.vector.bn_aggr(out=mvs[:, k, :], in_=stats[:])
                # stat2 col0 = mean, col1 = E[x^2] = var + mean^2 (batched)
                musq = tmpp.tile([P, KC], f32, tag="musq")
                nc.vector.tensor_tensor(musq[:], mvs[:, :, 0], mvs[:, :, 0], OP.mult)
                nc.vector.tensor_tensor(stat2[:, :, 1], mvs[:, :, 1], musq[:], OP.add)
                nc.vector.tensor_copy(out=stat2[:, :, 0], in_=mvs[:, :, 0])
                ps_g = qp.tile([P, 512], f32, tag="qp")
                for k in range(KC):
                    nc.tensor.matmul(
                        ps_g[0:GROUPS, 0:2], sel_sb[:, k, :], stat2[:, k, :],
                        start=(k == 0), stop=(k == KC - 1),
                    )
                # group mean / E2 -> rstd = exp(-0.5*ln(var+eps))
                gstat = big.tile([GROUPS, 2], f32, tag=f"gstat{uniq}")
                gms = tmpp.tile([GROUPS, 2], f32, tag="gms")
                nc.vector.tensor_copy(out=gms[:], in_=ps_g[0:GROUPS, 0:2])
                nc.vector.tensor_copy(out=gstat[:, 0:1], in_=gms[:, 0:1])
                gvar = tmpp.tile([GROUPS, 1], f32, tag="gvar")
                gmusq = tmpp.tile([GROUPS, 1], f32, tag="gmusq")
                nc.vector.tensor_tensor(gmusq[:], gms[:, 0:1], gms[:, 0:1], OP.mult)
                nc.vector.tensor_tensor(gvar[:], gms[:, 1:2], gmusq[:], OP.subtract)
                nc.scalar.activation(out=gvar[:], in_=gvar[:], func=AF.Ln, bias=eps_sb[:])
                nc.scalar.activation(out=gstat[:, 1:2], in_=gvar[:], func=AF.Exp, scale=-0.5)
                # broadcast (mean, rstd) back to channels, all chunks into one
                # psum, then batched A = scale*rstd, B = bias - mean*A
                ps_c = qp.tile([P, 512], f32, tag="qp")
                for k in range(KC):
                    nc.tensor.matmul(
                        ps_c[:, k * 2 : k * 2 + 2], selt_sb[:, k, :], gstat[:],
                        start=True, stop=True,
                    )
                cst = tmpp.tile([P, KC, 2], f32, tag="cst")
                nc.vector.tensor_copy(out=cst[:], in_=ps_c[:, 0 : 2 * KC])
                nc.vector.tensor_tensor(ab[:, :, 0], gs[:, :], cst[:, :, 1], OP.mult)
                ma = tmpp.tile([P, KC], f32, tag="ma")
                nc.vector.tensor_tensor(ma[:], cst[:, :, 0], ab[:, :, 0], OP.mult)
                nc.vector.tensor_tensor(ab[:, :, 1], gb[:, :], ma[:], OP.subtract)

            # ---------- GN1 -> hn ----------
            group_norm_stats(x_sb, g1s, g1b, ab1, "1")
            for k in range(KC):
                nc.vector.tensor_scalar(
                    hn[:, k, :], x_sb[:, k, :],
                    ab1[:, k, 0:1], ab1[:, k, 1:2], OP.mult, OP.add,
                )

            # ---------- phase helpers ----------
            def conv_out(dst_ap, w_sb, bias_sb, m):
                for t in range(NT):
                    ps = qp.tile([P, 512], f32, tag="qp")
                    for k in range(KC):
                        nc.tensor.matmul(
                            ps[:, :],
                            w_sb[:, m, k, :],
                            hn[:, k, t * 512 : (t + 1) * 512],
                            start=(k == 0), stop=(k == KC - 1),
                        )
                    nc.vector.tensor_scalar(
                        dst_ap[..., t * 512 : (t + 1) * 512], ps[:],
                        bias_sb[:, m : m + 1], None, OP.add,
                    )

            def v_tiles():
                # v^T tiles [s-part, head-major channel]; bv applied as an
                # extra K=1 contraction row (ones x bv) in the accumulation
                for nt in range(SC):
                    ps = qp.tile([P, 512], f32, tag="qp")
                    for k in range(KC):
                        nc.tensor.matmul(
                            ps[:, :],
                            hn[:, k, nt * P : (nt + 1) * P],
                            wv_sb[:, k, :],
                            start=(k == 0), stop=False,
                        )
                    nc.tensor.matmul(
                        ps[:, :],
                        ones1[:, 0:P],
                        bv_sb[:],
                        start=False, stop=True,
                    )
                    nc.vector.tensor_copy(
                        out=vt[:, nt, :, 0:CH],
                        in_=ps[:, :].rearrange("p (h c) -> p h c", h=NHEAD),
                    )

            a_u = big.tile([P, KC, N], f32r, tag="a_u")

            def head(h, qt, kt):
                p, e = h // 2, h % 2
                rows = slice(64 * e, 64 * e + 64)
                exps = expp.tile([P, SC, N], f32r, tag="exps")
                for sc in range(SC):
                    ps_st = stp.tile([P, N], f32, tag="stp")
                    for t in range(NT):
                        nc.tensor.matmul(
                            ps_st[:, t * 512 : (t + 1) * 512],
                            kt[rows, sc * P : (sc + 1) * P],
                            qt[rows, t * 512 : (t + 1) * 512],
                            start=True, stop=True,
                        )
                    nc.scalar.activation(out=exps[:, sc, :], in_=ps_st[:], func=AF.Exp)
                srow = 32 * p + e if p < 3 else 32 * e
                for t in range(NT):
                    ps_a = apool.tile([P, 512], f32, tag="apool")
                    for sc in range(SC):
                        nc.tensor.matmul(
                            ps_a[0 : CH + 1, :],
                            vt[:, sc, h, :],
                            exps[:, sc, t * 512 : (t + 1) * 512],
                            start=(sc == 0), stop=(sc == SC - 1),
                        )
                    tsl = slice(t * 512, (t + 1) * 512)
                    nc.vector.tensor_copy(out=a_u[rows, p, tsl], in_=ps_a[0:CH, :])
                    if e == 0 or p == 3:
                        # direct copy to a legal 32-aligned base
                        nc.vector.tensor_copy(
                            out=sums[srow : srow + 1, tsl], in_=ps_a[CH : CH + 1, :]
                        )
                    else:
                        nc.vector.tensor_copy(out=sums0[0:1, p, tsl], in_=ps_a[CH : CH + 1, :])
                if e == 1 and p < 3:
                    nc.sync.dma_start(sums[srow : srow + 1, :], sums0[0:1, p : p + 1, :])
                if debug_taps and h == 0:
                    nc.sync.dma_start(dbg["d_q"][:, 0, :], qt[:].bitcast(f32))
                    nc.sync.dma_start(dbg["d_exps0"][:], exps[:].bitcast(f32))

            # ---------- qkv, then attention ----------
            wp_sb = None

            def pair_recip(p):
                # per-pair softmax denominators: recip = exp(-ln(sums)).
                # Matmul operand bases must be 0/32/64, so pairs 0-2 sit at
                # partitions 32p; pair 3 uses rows 0 and 32 via eh region 1.
                if p < 3:
                    prow, reg = slice(32 * p, 32 * p + 2), 0
                else:
                    prow, reg = slice(0, 33), 1
                nc.scalar.activation(out=sums[prow, :], in_=sums[prow, :], func=AF.Ln)
                nc.scalar.activation(out=sums[prow, :], in_=sums[prow, :], func=AF.Exp, scale=-1.0)
                for t in range(NT):
                    ps_rb = apool.tile([P, 512], f32, tag="apool")
                    tsl = slice(t * 512, (t + 1) * 512)
                    nc.tensor.matmul(
                        ps_rb[:, :],
                        eh_sb[prow, reg, :],
                        sums[prow, tsl],
                        start=True, stop=True,
                    )
                    nc.vector.tensor_tensor(a_u[:, p, tsl], a_u[:, p, tsl], ps_rb[:], OP.mult)

            for p in range(KC):
                qt = qpool.tile([P, N], f32r, tag="qt")
                conv_out(qt[:, :], wq_sb, bq_sb, p)
                kt = kpool.tile([P, N], f32r, tag="kt")
                conv_out(kt[:, :], wk_sb, bk_sb, p)
                if p == 0:
                    v_tiles()
                    if debug_taps:
                        nc.sync.dma_start(dbg["d_hn"][:], hn[:].bitcast(f32))
                        nc.sync.dma_start(dbg["d_vt"][:], vt[:].bitcast(f32))
                if debug_taps:
                    nc.sync.dma_start(dbg["d_k"][:, p, :], kt[:].bitcast(f32))
                head(2 * p, qt, kt)
                head(2 * p + 1, qt, kt)
                if p == 0:
                    wp_sb = wpool.tile([P, KC, C], f32r, tag="w")
                    nc.sync.dma_start(wp_sb[:], wp_d[:])
                    nc.sync.dma_start(out_d[:], x_d[:])
                if p >= 1:
                    pair_recip(p - 1)
            pair_recip(3)

            if debug_taps:
                nc.sync.dma_start(dbg["d_au"][:], a_u[:].bitcast(f32))

            # ---------- proj ----------
            proj = vtp.tile([P, KC, N], f32r, tag="vp")  # vt is dead; same slot
            for m in range(KC):
                ps = stp.tile([P, N], f32, tag="stp")
                for t in range(NT):
                    for k in range(KC):
                        nc.tensor.matmul(
                            ps[:, t * 512 : (t + 1) * 512],
                            wp_sb[:, k, m * P : (m + 1) * P],
                            a_u[:, k, t * 512 : (t + 1) * 512],
                            start=(k == 0), stop=(k == KC - 1),
                        )
                nc.vector.tensor_scalar(
                    proj[:, m, :], ps[:], bp_sb[:, m : m + 1], None, OP.add,
                )

            if debug_taps:
                nc.sync.dma_start(dbg["d_proj"][:], proj[:].bitcast(f32))

            # ---------- GN2 + residual ----------
            group_norm_stats(proj, g2s, g2b, ab2, "2")
            for k in range(KC):
                nc.vector.tensor_scalar(
                    proj[:, k, :], proj[:, k, :],
                    ab2[:, k, 0:1], ab2[:, k, 1:2], OP.mult, OP.add,
                )
                nc.gpsimd.dma_start(
                    out_d[:, k, :], proj[:, k, :].bitcast(f32),
                    accum_op=OP.add,
                )

    nc.compile()
    return nc


def _host_prep(x, gn1_scale, gn1_bias, w_qkv, b_qkv, w_proj, b_proj, gn2_scale, gn2_bias):
    """Build per-core input maps (numpy only)."""
    f = np.float32
    x = np.asarray(x, f)
    w_qkv = np.asarray(w_qkv, f)
    b_qkv = np.asarray(b_qkv, f)
    w_proj = np.asarray(w_proj, f)
    b_proj = np.asarray(b_proj, f)
    gn1_scale = np.asarray(gn1_scale, f)
    gn1_bias = np.asarray(gn1_bias, f)
    gn2_scale = np.asarray(gn2_scale, f)
    gn2_bias = np.asarray(gn2_bias, f)

    def chunk_vec(v):  # [C] -> [P, KC]
        return np.ascontiguousarray(v.reshape(KC, P).T)

    def chunk_mat(wt):  # [C, M] -> [P, KC, M]
        return np.ascontiguousarray(wt.reshape(KC, P, -1).transpose(1, 0, 2))

    idx = np.arange(NHEAD)[:, None] * (3 * CH) + np.arange(CH)[None, :]
    q_idx, k_idx, v_idx = idx.ravel(), (idx + CH).ravel(), (idx + 2 * CH).ravel()

    s2 = float(CH) ** -0.5
    def mtile(w):  # [P, KC, C] -> [P, M, KC, P]
        return np.ascontiguousarray(
            w.reshape(P, KC, KC, P).transpose(0, 2, 1, 3)
        )
    wq = mtile(chunk_mat(w_qkv[q_idx].T * s2))
    wk = mtile(chunk_mat(w_qkv[k_idx].T))
    wv = chunk_mat(w_qkv[v_idx].T)
    wp = chunk_mat(w_proj.T)
    bq = chunk_vec(b_qkv[q_idx] * s2)
    bk = chunk_vec(b_qkv[k_idx])
    bv = np.ascontiguousarray(b_qkv[v_idx].reshape(1, C))
    bp = chunk_vec(b_proj)

    cidx = np.arange(C)
    sel = np.zeros((P, KC, GROUPS), f)
    sel[cidx % P, cidx // P, cidx // 16] = 1.0 / 16.0
    selt = np.zeros((GROUPS, KC, P), f)
    selt[cidx // 16, cidx // P, cidx % P] = 1.0
    eh = np.zeros((P, 2, P), f)
    for pp in range(3):
        for c in range(P):
            eh[32 * pp + c // CH, 0, c] = 1.0
    for c in range(P):
        eh[32 * (c // CH), 1, c] = 1.0

    shared = {
        "wq": wq, "wk": wk, "wv": wv, "wp": wp,
        "bq": bq, "bk": bk, "bv": bv, "bp": bp,
        "g1s": chunk_vec(gn1_scale), "g1b": chunk_vec(gn1_bias),
        "g2s": chunk_vec(gn2_scale), "g2b": chunk_vec(gn2_bias),
        "sel": sel, "selt": selt, "eh": eh,
    }
    in_maps = []
    for b in range(B):
        xb = np.ascontiguousarray(
            x[b].reshape(C, N).reshape(KC, P, N).transpose(1, 0, 2)
        )
        in_maps.append({"x": xb, **shared})
    return in_maps


def _assemble(results):
    out = np.empty((B, C, H, W), np.float32)
    for b in range(B):
        ob = np.asarray(results[b]["out"])  # [P, KC, N]
        out[b] = ob.transpose(1, 0, 2).reshape(C, N).reshape(C, H, W)
    return out


def get_nc():
    if "nc" not in _CACHE:
        _CACHE["nc"] = _build_nc()
    return _CACHE["nc"]


def kernel(x, gn1_scale, gn1_bias, w_qkv, b_qkv, w_proj, b_proj, gn2_scale, gn2_bias):
    from concourse.bass_utils import run_bass_kernel_spmd

    nc = get_nc()
    in_maps = _host_prep(
        x, gn1_scale, gn1_bias, w_qkv, b_qkv, w_proj, b_proj, gn2_scale, gn2_bias
    )
    res = run_bass_kernel_spmd(nc, in_maps, core_ids=list(range(B)))
    return _assemble(res.results)
